# revision 4
# baseline (speedup 1.0000x reference)
"""Multi-head attention (B=2, S=2048, d_model=1024, 16 heads) on 8 trn2 cores.

Sharding: core c -> batch c//4, head-group c%4 (4 heads, 256 feature cols).
Per-core kernel computes, for its batch's tokens x and its 256-col slices of
Wq/Wk/Wv (col-parallel) and 256-row slice of Wo (row-parallel):
  QT/KT = (x @ W + b).T   [256, 2048]  (feature-major, via f32r matmuls)
  V     = x @ Wv + bv     [2048, 256]  (seq-major)
  per head: scores = Q K^T / 8 -> exp (no max-sub; scores are O(5)) with
  fused row-sum -> normalize (gpsimd) -> attn out + PE-transpose -> attnT
  -> ctxT = V^T @ attnT -> out_part = ctx @ Wo  (bias bo added on host)
Host sums the 4 per-batch out_part partials and concatenates attn heads.
"""

import os
import sys

for _p in ("/opt/trn_rl_repo", "/opt/pypackages"):
    if os.path.isdir(_p) and _p not in sys.path:
        sys.path.insert(0, _p)

import numpy as np

import concourse.bass as bass
import concourse.tile as tile
from concourse import bacc, mybir
from concourse.bass_utils import run_bass_kernel_spmd
from concourse.masks import make_identity

P = 128
S = 2048
D_MODEL = 1024
NHEAD = 16
DK = 64
HPC = 4            # heads per core
DPC = HPC * DK     # 256: feature columns per core
KT_TILES = D_MODEL // P   # 8 k-tiles over the contraction dim
ST_TILES = S // P         # 16 tiles over the sequence
F32 = mybir.dt.float32
F32R = mybir.dt.float32r
AF = mybir.ActivationFunctionType


def _build_nc():
    nc = bacc.Bacc("TRN2", target_bir_lowering=False, debug=False)

    xqT_d = nc.dram_tensor("xqT", [D_MODEL, S], F32R, kind="ExternalInput").ap()
    xkT_d = nc.dram_tensor("xkT", [D_MODEL, S], F32R, kind="ExternalInput").ap()
    xvT_d = nc.dram_tensor("xvT", [D_MODEL, S], F32R, kind="ExternalInput").ap()
    wq_d = nc.dram_tensor("wq", [D_MODEL, DPC], F32R, kind="ExternalInput").ap()
    wk_d = nc.dram_tensor("wk", [D_MODEL, DPC], F32R, kind="ExternalInput").ap()
    wv_d = nc.dram_tensor("wv", [D_MODEL, DPC], F32R, kind="ExternalInput").ap()
    wo_d = nc.dram_tensor("wo", [DPC, D_MODEL], F32R, kind="ExternalInput").ap()
    bq_d = nc.dram_tensor("bq", [1, DPC], F32R, kind="ExternalInput").ap()
    bk_d = nc.dram_tensor("bk", [1, DPC], F32R, kind="ExternalInput").ap()
    bv_d = nc.dram_tensor("bv", [1, DPC], F32R, kind="ExternalInput").ap()
    attn_d = nc.dram_tensor("attn_o", [HPC, S, S], F32R, kind="ExternalOutput").ap()
    out_d = nc.dram_tensor("out_o", [S, D_MODEL], F32, kind="ExternalOutput").ap()

    with tile.TileContext(nc) as tc:
        with (
            tc.tile_pool(name="cpool", bufs=1) as cpool,
            tc.tile_pool(name="xpool", bufs=8) as xpool,
            tc.tile_pool(name="qkv", bufs=1) as qkv,
            tc.tile_pool(name="work", bufs=2) as work,
            tc.tile_pool(name="stats", bufs=8) as stats,
            tc.tile_pool(name="ps_big", bufs=2, space="PSUM") as ps_big,
            tc.tile_pool(name="ps_tp", bufs=2, space="PSUM") as ps_tp,
            tc.tile_pool(name="ps_acc", bufs=2, space="PSUM") as ps_acc,
        ):
            ident_f = cpool.tile([P, P], F32)
            make_identity(nc, ident_f)
            ident = cpool.tile([P, P], F32R)
            nc.any.tensor_copy(ident, ident_f)
            ones_f = cpool.tile([1, 512], F32)
            nc.gpsimd.memset(ones_f, 1.0)
            ones = cpool.tile([1, 512], F32R)
            nc.any.tensor_copy(ones, ones_f)
            zbias = cpool.tile([P, 1], F32)
            nc.gpsimd.memset(zbias, 0.0)

            wq_sb = cpool.tile([P, KT_TILES, DPC], F32R)
            nc.sync.dma_start(wq_sb, wq_d.rearrange("(kt p) m -> p kt m", p=P))
            wk_sb = cpool.tile([P, KT_TILES, DPC], F32R)
            nc.sync.dma_start(wk_sb, wk_d.rearrange("(kt p) m -> p kt m", p=P))
            wv_sb = cpool.tile([P, KT_TILES, DPC], F32R)
            nc.sync.dma_start(wv_sb, wv_d.rearrange("(kt p) m -> p kt m", p=P))
            wo_sb = cpool.tile([P, 2, D_MODEL], F32R)
            nc.sync.dma_start(wo_sb, wo_d.rearrange("(kt p) m -> p kt m", p=P))
            bq_sb = cpool.tile([1, DPC], F32R)
            nc.sync.dma_start(bq_sb, bq_d)
            bk_sb = cpool.tile([1, DPC], F32R)
            nc.sync.dma_start(bk_sb, bk_d)
            bv_sb = cpool.tile([1, DPC], F32R)
            nc.sync.dma_start(bv_sb, bv_d)

            QT = qkv.tile([P, 2, S], F32R)   # [d%128, d//128, q]
            KT = qkv.tile([P, 2, S], F32R)
            Vsb = qkv.tile([P, ST_TILES, DPC], F32R)  # [s%128, s//128, d]
            ctxT = qkv.tile([P, 2, S], F32R)

            # ---- Q/K projections, feature-major: dst[d, q] = W.T @ xT + b ----
            for xdram, w_sb, b_sb, dst in (
                (xqT_d, wq_sb, bq_sb, QT),
                (xkT_d, wk_sb, bk_sb, KT),
            ):
                xts = []
                for kt in range(KT_TILES):
                    xt_t = xpool.tile([P, S], F32R, tag="b8k", name=f"xt{kt}")
                    nc.sync.dma_start(xt_t, xdram[kt * P : (kt + 1) * P, :])
                    xts.append(xt_t)
                for m in range(2):
                    for nn in range(2):
                        ps = ps_big.tile([P, 1024], F32, tag="big", name="proj_ps")
                        for kt in range(KT_TILES):
                            for sub in range(2):
                                c0 = nn * 1024 + sub * 512
                                nc.tensor.matmul(
                                    ps[:, sub * 512 : (sub + 1) * 512],
                                    w_sb[:, kt, m * P : (m + 1) * P],
                                    xts[kt][:, c0 : c0 + 512],
                                    start=(kt == 0),
                                    stop=False,
                                )
                        for sub in range(2):
                            nc.tensor.matmul(
                                ps[:, sub * 512 : (sub + 1) * 512],
                                b_sb[:, m * P : (m + 1) * P],
                                ones[:, :512],
                                start=False,
                                stop=True,
                            )
                        nc.any.tensor_copy(
                            dst[:, m, nn * 1024 : (nn + 1) * 1024], ps
                        )

            # ---- V projection, seq-major: V[s, d] = xT.T @ Wv + bv ----
            xts = []
            for kt in range(KT_TILES):
                xt_t = xpool.tile([P, S], F32R, tag="b8k", name=f"xv{kt}")
                nc.sync.dma_start(xt_t, xvT_d[kt * P : (kt + 1) * P, :])
                xts.append(xt_t)
            for st in range(ST_TILES):
                ps = ps_big.tile([P, 1024], F32, tag="big", name="v_ps")
                psv = ps[:, :DPC]
                for kt in range(KT_TILES):
                    nc.tensor.matmul(
                        psv,
                        xts[kt][:, st * P : (st + 1) * P],
                        wv_sb[:, kt, :],
                        start=(kt == 0),
                        stop=False,
                    )
                nc.tensor.matmul(
                    psv, ones[:, :P], bv_sb, start=False, stop=True
                )
                nc.any.tensor_copy(Vsb[:, st, :], psv)

            # ---- attention per head ----
            for h in range(HPC):
                ti, pr = h // 2, (h % 2) * DK
                att_tiles = None
                for qt in range(ST_TILES):
                    ex = work.tile([P, S], F32R, tag="ex", name="ex")
                    rs2 = stats.tile([P, 2], F32, tag="rs2", name="rs2")
                    for half in range(2):
                        ps = ps_big.tile([P, 1024], F32, tag="big", name="sc_ps")
                        for sub in range(2):
                            col = half * 1024 + sub * 512
                            nc.tensor.matmul(
                                ps[:, sub * 512 : (sub + 1) * 512],
                                QT[pr : pr + DK, ti, qt * P : (qt + 1) * P],
                                KT[pr : pr + DK, ti, col : col + 512],
                                start=True,
                                stop=True,
                            )
                        nc.scalar.activation(
                            ex[:, half * 1024 : (half + 1) * 1024],
                            ps,
                            AF.Exp,
                            bias=zbias,
                            scale=0.125,
                            accum_out=rs2[:, half : half + 1],
                        )
                    rsum = stats.tile([P, 1], F32, tag="rs1", name="rsum")
                    nc.vector.reduce_sum(rsum, rs2, axis=mybir.AxisListType.X)
                    inv = stats.tile([P, 1], F32, tag="inv", name="inv")
                    nc.vector.reciprocal(inv, rsum)
                    nc.gpsimd.tensor_scalar_mul(ex, ex, inv)
                    nc.sync.dma_start(attn_d[h, qt * P : (qt + 1) * P, :], ex)

                    # transpose the normalized 128-q row-block into attnT tiles
                    if qt % 4 == 0:
                        att_tiles = [
                            xpool.tile([P, 4, 512], F32R, tag="b8k", name=f"attT{jj}")
                            for jj in range(4)
                        ]
                    c0 = (qt % 4) * P
                    for jj in range(4):
                        tp = ps_tp.tile([P, 512], F32R, tag="tp", name="tp")
                        for j2 in range(4):
                            j = jj * 4 + j2
                            nc.tensor.transpose(
                                tp[:, j2 * P : (j2 + 1) * P],
                                ex[:, j * P : (j + 1) * P],
                                ident,
                            )
                        nc.any.tensor_copy(
                            att_tiles[jj][:, :, c0 : c0 + P],
                            tp.rearrange("p (a b) -> p a b", a=4),
                        )

                    if qt % 4 == 3:
                        qc = qt // 4
                        cps = ps_acc.tile([DK, 512], F32, tag="acc", name="cps")
                        for j in range(ST_TILES):
                            nc.tensor.matmul(
                                cps,
                                Vsb[:, j, h * DK : (h + 1) * DK],
                                att_tiles[j // 4][:, j % 4, :],
                                start=(j == 0),
                                stop=(j == ST_TILES - 1),
                            )
                        nc.any.tensor_copy(
                            ctxT[pr : pr + DK, ti, qc * 512 : (qc + 1) * 512], cps
                        )

            # ---- output projection: out[q, :] = ctx @ Wo ----
            for qt in range(ST_TILES):
                ob = work.tile([P, D_MODEL], F32, tag="ob", name="ob")
                for nco in range(2):
                    ps = ps_acc.tile([P, 512], F32, tag="acc", name="o_ps")
                    for kt2 in range(2):
                        nc.tensor.matmul(
                            ps,
                            ctxT[:, kt2, qt * P : (qt + 1) * P],
                            wo_sb[:, kt2, nco * 512 : (nco + 1) * 512],
                            start=(kt2 == 0),
                            stop=(kt2 == 1),
                        )
                    nc.any.tensor_copy(ob[:, nco * 512 : (nco + 1) * 512], ps)
                nc.sync.dma_start(out_d[qt * P : (qt + 1) * P, :], ob)

    nc.compile()
    return nc


_NC_CACHE = {}


def _get_nc():
    if "nc" not in _NC_CACHE:
        _NC_CACHE["nc"] = _build_nc()
    return _NC_CACHE["nc"]


def make_in_maps(query, key, value, Wq, bq, Wk, bk, Wv, bv, Wo, bo=None):
    query = np.asarray(query, np.float32)
    key = np.asarray(key, np.float32)
    value = np.asarray(value, np.float32)
    Wq = np.asarray(Wq, np.float32)
    Wk = np.asarray(Wk, np.float32)
    Wv = np.asarray(Wv, np.float32)
    Wo = np.asarray(Wo, np.float32)
    bq = np.asarray(bq, np.float32)
    bk = np.asarray(bk, np.float32)
    bv = np.asarray(bv, np.float32)

    xT = {}
    for b in range(2):
        xT[b] = (
            np.ascontiguousarray(query[b].T),
            np.ascontiguousarray(key[b].T),
            np.ascontiguousarray(value[b].T),
        )
    in_maps = []
    for c in range(8):
        b, g = divmod(c, 4)
        ds = slice(g * DPC, (g + 1) * DPC)
        xq, xk, xv = xT[b]
        in_maps.append(
            {
                "xqT": xq,
                "xkT": xk,
                "xvT": xv,
                "wq": np.ascontiguousarray(Wq[:, ds]),
                "wk": np.ascontiguousarray(Wk[:, ds]),
                "wv": np.ascontiguousarray(Wv[:, ds]),
                "wo": np.ascontiguousarray(Wo[ds, :]),
                "bq": np.ascontiguousarray(bq[ds]).reshape(1, DPC),
                "bk": np.ascontiguousarray(bk[ds]).reshape(1, DPC),
                "bv": np.ascontiguousarray(bv[ds]).reshape(1, DPC),
            }
        )
    return in_maps


def assemble(results, bo):
    bo = np.asarray(bo, np.float32)
    attn = np.empty((2, NHEAD, S, S), np.float32)
    out = np.zeros((2, S, D_MODEL), np.float32)
    for c in range(8):
        b, g = divmod(c, 4)
        attn[b, g * HPC : (g + 1) * HPC] = results[c]["attn_o"]
        out[b] += results[c]["out_o"]
    out += bo
    return out, attn


def run(trace=False, trace_cores=None, **inputs):
    nc = _get_nc()
    in_maps = make_in_maps(**{k: v for k, v in inputs.items() if k != "bo"})
    res = run_bass_kernel_spmd(
        nc,
        in_maps,
        core_ids=list(range(8)),
        trace=trace,
        trace_cores=trace_cores,
    )
    out, attn = assemble(res.results, inputs["bo"])
    return out, attn, res


def kernel(query, key, value, Wq, bq, Wk, bk, Wv, bv, Wo, bo):
    out, attn, _ = run(
        query=query, key=key, value=value, Wq=Wq, bq=bq, Wk=Wk, bk=bk,
        Wv=Wv, bv=bv, Wo=Wo, bo=bo,
    )
    return out, attn


# revision 5
# speedup vs baseline: 3.9575x; 3.9575x over previous
"""Multi-head attention (B=2, S=2048, d_model=1024, 16 heads) on 8 trn2 cores.

Sharding: core c -> batch c//4, head-group c%4 (4 heads, 256 feature cols).
Per-core kernel computes, for its batch's tokens x and its 256-col slices of
Wq/Wk/Wv (col-parallel) and 256-row slice of Wo (row-parallel):
  QT/KT = (x @ W + b).T   [256, 2048]  (feature-major, via f32r matmuls)
  V     = x @ Wv + bv     [2048, 256]  (seq-major)
  per head: scores = Q K^T / 8 -> exp (no max-sub; scores are O(5)) with
  fused row-sum -> normalize (gpsimd) -> attn out + PE-transpose -> attnT
  -> ctxT = V^T @ attnT -> out_part = ctx @ Wo  (bias bo added on host)
Host sums the 4 per-batch out_part partials and concatenates attn heads.
"""

import os
import sys

for _p in ("/opt/trn_rl_repo", "/opt/pypackages"):
    if os.path.isdir(_p) and _p not in sys.path:
        sys.path.insert(0, _p)

import numpy as np

import concourse.bass as bass
import concourse.tile as tile
from concourse import bacc, mybir
from concourse.bass_utils import run_bass_kernel_spmd
from concourse.masks import make_identity

P = 128
S = 2048
D_MODEL = 1024
NHEAD = 16
DK = 64
HPC = 4            # heads per core
DPC = HPC * DK     # 256: feature columns per core
KT_TILES = D_MODEL // P   # 8 k-tiles over the contraction dim
ST_TILES = S // P         # 16 tiles over the sequence
F32 = mybir.dt.float32
F32R = mybir.dt.float32r
AF = mybir.ActivationFunctionType


def _build_nc():
    nc = bacc.Bacc("TRN2", target_bir_lowering=False, debug=False)

    xqT_d = nc.dram_tensor("xqT", [D_MODEL, S], F32R, kind="ExternalInput").ap()
    xkT_d = nc.dram_tensor("xkT", [D_MODEL, S], F32R, kind="ExternalInput").ap()
    xvT_d = nc.dram_tensor("xvT", [D_MODEL, S], F32R, kind="ExternalInput").ap()
    wq_d = nc.dram_tensor("wq", [D_MODEL, DPC], F32R, kind="ExternalInput").ap()
    wk_d = nc.dram_tensor("wk", [D_MODEL, DPC], F32R, kind="ExternalInput").ap()
    wv_d = nc.dram_tensor("wv", [D_MODEL, DPC], F32R, kind="ExternalInput").ap()
    wo_d = nc.dram_tensor("wo", [DPC, D_MODEL], F32R, kind="ExternalInput").ap()
    bq_d = nc.dram_tensor("bq", [1, DPC], F32R, kind="ExternalInput").ap()
    bk_d = nc.dram_tensor("bk", [1, DPC], F32R, kind="ExternalInput").ap()
    bv_d = nc.dram_tensor("bv", [1, DPC], F32R, kind="ExternalInput").ap()
    attn_d = nc.dram_tensor("attn_o", [HPC, S, S], F32R, kind="ExternalOutput").ap()
    out_d = nc.dram_tensor("out_o", [S, D_MODEL], F32, kind="ExternalOutput").ap()

    with tile.TileContext(nc) as tc:
        with (
            tc.tile_pool(name="cpool", bufs=1) as cpool,
            tc.tile_pool(name="xpool", bufs=8) as xpool,
            tc.tile_pool(name="qkv", bufs=1) as qkv,
            tc.tile_pool(name="work", bufs=2) as work,
            tc.tile_pool(name="stats", bufs=8) as stats,
            tc.tile_pool(name="ps_big", bufs=2, space="PSUM") as ps_big,
            tc.tile_pool(name="ps_tp", bufs=2, space="PSUM") as ps_tp,
            tc.tile_pool(name="ps_acc", bufs=2, space="PSUM") as ps_acc,
        ):
            ident_f = cpool.tile([P, P], F32)
            make_identity(nc, ident_f)
            ident = cpool.tile([P, P], F32R)
            nc.any.tensor_copy(ident, ident_f)
            ones_f = cpool.tile([1, 512], F32)
            nc.gpsimd.memset(ones_f, 1.0)
            ones = cpool.tile([1, 512], F32R)
            nc.any.tensor_copy(ones, ones_f)
            zbias = cpool.tile([P, 1], F32)
            nc.gpsimd.memset(zbias, 0.0)

            wq_sb = cpool.tile([P, KT_TILES, DPC], F32R)
            nc.sync.dma_start(wq_sb, wq_d.rearrange("(kt p) m -> p kt m", p=P))
            wk_sb = cpool.tile([P, KT_TILES, DPC], F32R)
            nc.sync.dma_start(wk_sb, wk_d.rearrange("(kt p) m -> p kt m", p=P))
            wv_sb = cpool.tile([P, KT_TILES, DPC], F32R)
            nc.sync.dma_start(wv_sb, wv_d.rearrange("(kt p) m -> p kt m", p=P))
            wo_sb = cpool.tile([P, 2, D_MODEL], F32R)
            nc.sync.dma_start(wo_sb, wo_d.rearrange("(kt p) m -> p kt m", p=P))
            bq_sb = cpool.tile([1, DPC], F32R)
            nc.sync.dma_start(bq_sb, bq_d)
            bk_sb = cpool.tile([1, DPC], F32R)
            nc.sync.dma_start(bk_sb, bk_d)
            bv_sb = cpool.tile([1, DPC], F32R)
            nc.sync.dma_start(bv_sb, bv_d)

            QT = qkv.tile([P, 2, S], F32R)   # [d%128, d//128, q]
            KT = qkv.tile([P, 2, S], F32R)
            Vsb = qkv.tile([P, ST_TILES, DPC], F32R)  # [s%128, s//128, d]
            ctxT = qkv.tile([P, 2, S], F32R)

            # ---- Q/K projections, feature-major: dst[d, q] = W.T @ xT + b ----
            for xdram, w_sb, b_sb, dst in (
                (xqT_d, wq_sb, bq_sb, QT),
                (xkT_d, wk_sb, bk_sb, KT),
            ):
                xts = []
                for kt in range(KT_TILES):
                    xt_t = xpool.tile([P, S], F32R, tag="b8k", name=f"xt{kt}")
                    nc.sync.dma_start(xt_t, xdram[kt * P : (kt + 1) * P, :])
                    xts.append(xt_t)
                for m in range(2):
                    for nn in range(2):
                        ps = ps_big.tile([P, 1024], F32, tag="big", name="proj_ps")
                        for kt in range(KT_TILES):
                            for sub in range(2):
                                c0 = nn * 1024 + sub * 512
                                nc.tensor.matmul(
                                    ps[:, sub * 512 : (sub + 1) * 512],
                                    w_sb[:, kt, m * P : (m + 1) * P],
                                    xts[kt][:, c0 : c0 + 512],
                                    start=(kt == 0),
                                    stop=False,
                                )
                        for sub in range(2):
                            nc.tensor.matmul(
                                ps[:, sub * 512 : (sub + 1) * 512],
                                b_sb[:, m * P : (m + 1) * P],
                                ones[:, :512],
                                start=False,
                                stop=True,
                            )
                        nc.any.tensor_copy(
                            dst[:, m, nn * 1024 : (nn + 1) * 1024], ps
                        )

            # ---- V projection, seq-major: V[s, d] = xT.T @ Wv + bv ----
            xts = []
            for kt in range(KT_TILES):
                xt_t = xpool.tile([P, S], F32R, tag="b8k", name=f"xv{kt}")
                nc.sync.dma_start(xt_t, xvT_d[kt * P : (kt + 1) * P, :])
                xts.append(xt_t)
            for st in range(ST_TILES):
                ps = ps_big.tile([P, 1024], F32, tag="big", name="v_ps")
                psv = ps[:, :DPC]
                for kt in range(KT_TILES):
                    nc.tensor.matmul(
                        psv,
                        xts[kt][:, st * P : (st + 1) * P],
                        wv_sb[:, kt, :],
                        start=(kt == 0),
                        stop=False,
                    )
                nc.tensor.matmul(
                    psv, ones[:, :P], bv_sb, start=False, stop=True
                )
                nc.any.tensor_copy(Vsb[:, st, :], psv)

            # ---- attention per head ----
            for h in range(HPC):
                ti, pr = h // 2, (h % 2) * DK
                att_tiles = None
                for qt in range(ST_TILES):
                    ex = work.tile([P, S], F32R, tag="ex", name="ex")
                    rs2 = stats.tile([P, 2], F32, tag="rs2", name="rs2")
                    for half in range(2):
                        ps = ps_big.tile([P, 1024], F32, tag="big", name="sc_ps")
                        for sub in range(2):
                            col = half * 1024 + sub * 512
                            nc.tensor.matmul(
                                ps[:, sub * 512 : (sub + 1) * 512],
                                QT[pr : pr + DK, ti, qt * P : (qt + 1) * P],
                                KT[pr : pr + DK, ti, col : col + 512],
                                start=True,
                                stop=True,
                            )
                        nc.scalar.activation(
                            ex[:, half * 1024 : (half + 1) * 1024],
                            ps,
                            AF.Exp,
                            bias=zbias,
                            scale=0.125,
                            accum_out=rs2[:, half : half + 1],
                        )
                    rsum = stats.tile([P, 1], F32, tag="rs1", name="rsum")
                    nc.vector.reduce_sum(rsum, rs2, axis=mybir.AxisListType.X)
                    inv = stats.tile([P, 1], F32, tag="inv", name="inv")
                    nc.vector.reciprocal(inv, rsum)
                    nc.vector.tensor_scalar_mul(ex, ex, inv)
                    nc.sync.dma_start(attn_d[h, qt * P : (qt + 1) * P, :], ex)

                    # transpose the normalized 128-q row-block into attnT tiles
                    if qt % 4 == 0:
                        att_tiles = [
                            xpool.tile([P, 4, 512], F32R, tag="b8k", name=f"attT{jj}")
                            for jj in range(4)
                        ]
                    c0 = (qt % 4) * P
                    for jj in range(4):
                        tp = ps_tp.tile([P, 512], F32R, tag="tp", name="tp")
                        for j2 in range(4):
                            j = jj * 4 + j2
                            nc.tensor.transpose(
                                tp[:, j2 * P : (j2 + 1) * P],
                                ex[:, j * P : (j + 1) * P],
                                ident,
                            )
                        nc.any.tensor_copy(
                            att_tiles[jj][:, :, c0 : c0 + P],
                            tp.rearrange("p (a b) -> p a b", a=4),
                        )

                    if qt % 4 == 3:
                        qc = qt // 4
                        cps = ps_acc.tile([DK, 512], F32, tag="acc", name="cps")
                        for j in range(ST_TILES):
                            nc.tensor.matmul(
                                cps,
                                Vsb[:, j, h * DK : (h + 1) * DK],
                                att_tiles[j // 4][:, j % 4, :],
                                start=(j == 0),
                                stop=(j == ST_TILES - 1),
                            )
                        nc.any.tensor_copy(
                            ctxT[pr : pr + DK, ti, qc * 512 : (qc + 1) * 512], cps
                        )

            # ---- output projection: out[q, :] = ctx @ Wo ----
            for qt in range(ST_TILES):
                ob = work.tile([P, D_MODEL], F32, tag="ob", name="ob")
                for nco in range(2):
                    ps = ps_acc.tile([P, 512], F32, tag="acc", name="o_ps")
                    for kt2 in range(2):
                        nc.tensor.matmul(
                            ps,
                            ctxT[:, kt2, qt * P : (qt + 1) * P],
                            wo_sb[:, kt2, nco * 512 : (nco + 1) * 512],
                            start=(kt2 == 0),
                            stop=(kt2 == 1),
                        )
                    nc.any.tensor_copy(ob[:, nco * 512 : (nco + 1) * 512], ps)
                nc.sync.dma_start(out_d[qt * P : (qt + 1) * P, :], ob)

    nc.compile()
    return nc


_NC_CACHE = {}


def _get_nc():
    if "nc" not in _NC_CACHE:
        _NC_CACHE["nc"] = _build_nc()
    return _NC_CACHE["nc"]


def make_in_maps(query, key, value, Wq, bq, Wk, bk, Wv, bv, Wo, bo=None):
    query = np.asarray(query, np.float32)
    key = np.asarray(key, np.float32)
    value = np.asarray(value, np.float32)
    Wq = np.asarray(Wq, np.float32)
    Wk = np.asarray(Wk, np.float32)
    Wv = np.asarray(Wv, np.float32)
    Wo = np.asarray(Wo, np.float32)
    bq = np.asarray(bq, np.float32)
    bk = np.asarray(bk, np.float32)
    bv = np.asarray(bv, np.float32)

    xT = {}
    for b in range(2):
        xT[b] = (
            np.ascontiguousarray(query[b].T),
            np.ascontiguousarray(key[b].T),
            np.ascontiguousarray(value[b].T),
        )
    in_maps = []
    for c in range(8):
        b, g = divmod(c, 4)
        ds = slice(g * DPC, (g + 1) * DPC)
        xq, xk, xv = xT[b]
        in_maps.append(
            {
                "xqT": xq,
                "xkT": xk,
                "xvT": xv,
                "wq": np.ascontiguousarray(Wq[:, ds]),
                "wk": np.ascontiguousarray(Wk[:, ds]),
                "wv": np.ascontiguousarray(Wv[:, ds]),
                "wo": np.ascontiguousarray(Wo[ds, :]),
                "bq": np.ascontiguousarray(bq[ds]).reshape(1, DPC),
                "bk": np.ascontiguousarray(bk[ds]).reshape(1, DPC),
                "bv": np.ascontiguousarray(bv[ds]).reshape(1, DPC),
            }
        )
    return in_maps


def assemble(results, bo):
    bo = np.asarray(bo, np.float32)
    attn = np.empty((2, NHEAD, S, S), np.float32)
    out = np.zeros((2, S, D_MODEL), np.float32)
    for c in range(8):
        b, g = divmod(c, 4)
        attn[b, g * HPC : (g + 1) * HPC] = results[c]["attn_o"]
        out[b] += results[c]["out_o"]
    out += bo
    return out, attn


def run(trace=False, trace_cores=None, **inputs):
    nc = _get_nc()
    in_maps = make_in_maps(**{k: v for k, v in inputs.items() if k != "bo"})
    res = run_bass_kernel_spmd(
        nc,
        in_maps,
        core_ids=list(range(8)),
        trace=trace,
        trace_cores=trace_cores,
    )
    out, attn = assemble(res.results, inputs["bo"])
    return out, attn, res


def kernel(query, key, value, Wq, bq, Wk, bk, Wv, bv, Wo, bo):
    out, attn, _ = run(
        query=query, key=key, value=value, Wq=Wq, bq=bq, Wk=Wk, bk=bk,
        Wv=Wv, bv=bv, Wo=Wo, bo=bo,
    )
    return out, attn


# revision 7
# speedup vs baseline: 4.3017x; 1.0870x over previous
"""Multi-head attention (B=2, S=2048, d_model=1024, 16 heads) on 8 trn2 cores.

Sharding: core c -> batch c//4, head-group c%4 (4 heads, 256 feature cols).
Per-core kernel, for its batch's tokens x and its 256-col slices of Wq/Wk/Wv
(col-parallel) and 256-row slice of Wo (row-parallel):
  QT/KT = (x @ W + b).T   [256, 2048]  f32r (tf32 matmuls, fp32 accumulate)
  V     = x @ Wv + bv     [2048, 256]  bf16 seq-major
  per head: scores = Q K^T (f32r, K=64) -> exp(x/8) on ACT with fused row-sum
  (no max-sub; scores are O(5)), bf16 exp tile -> attn = exp * (1/rsum) (DVE,
  bf16, DMA'd out) while the *unnormalized* bf16 exp is PE-transposed ->
  ctx_exp^T = V^T @ exp^T -> scaled by a PE-broadcast 1/rsum row -> ctxT f32r
  -> out_part = ctx @ Wo (f32 out; bias bo added on host).
Host sums the 4 per-batch out_part partials and concatenates attn heads.
"""

import os
import sys

for _p in ("/opt/trn_rl_repo", "/opt/pypackages"):
    if os.path.isdir(_p) and _p not in sys.path:
        sys.path.insert(0, _p)

import numpy as np

import concourse.bass as bass
import concourse.tile as tile
from concourse import bacc, mybir
from concourse.bass_utils import run_bass_kernel_spmd
from concourse.masks import make_identity

P = 128
S = 2048
D_MODEL = 1024
NHEAD = 16
DK = 64
HPC = 4            # heads per core
DPC = HPC * DK     # 256: feature columns per core
KT_TILES = D_MODEL // P   # 8 k-tiles over the contraction dim
ST_TILES = S // P         # 16 tiles over the sequence
F32 = mybir.dt.float32
F32R = mybir.dt.float32r
BF16 = mybir.dt.bfloat16
AF = mybir.ActivationFunctionType


def _build_nc():
    nc = bacc.Bacc("TRN2", target_bir_lowering=False, debug=False)

    xqT_d = nc.dram_tensor("xqT", [D_MODEL, S], F32R, kind="ExternalInput").ap()
    xkT_d = nc.dram_tensor("xkT", [D_MODEL, S], F32R, kind="ExternalInput").ap()
    xvT_d = nc.dram_tensor("xvT", [D_MODEL, S], F32R, kind="ExternalInput").ap()
    wq_d = nc.dram_tensor("wq", [D_MODEL, DPC], F32R, kind="ExternalInput").ap()
    wk_d = nc.dram_tensor("wk", [D_MODEL, DPC], F32R, kind="ExternalInput").ap()
    wv_d = nc.dram_tensor("wv", [D_MODEL, DPC], F32R, kind="ExternalInput").ap()
    wo_d = nc.dram_tensor("wo", [DPC, D_MODEL], F32R, kind="ExternalInput").ap()
    bq_d = nc.dram_tensor("bq", [1, DPC], F32R, kind="ExternalInput").ap()
    bk_d = nc.dram_tensor("bk", [1, DPC], F32R, kind="ExternalInput").ap()
    bv_d = nc.dram_tensor("bv", [1, DPC], F32R, kind="ExternalInput").ap()
    attn_d = nc.dram_tensor("attn_o", [HPC, S, S], BF16, kind="ExternalOutput").ap()
    out_d = nc.dram_tensor("out_o", [S, D_MODEL], F32, kind="ExternalOutput").ap()

    with tile.TileContext(nc) as tc:
        with (
            tc.tile_pool(name="cpool", bufs=1) as cpool,
            tc.tile_pool(name="xpool", bufs=8) as xpool,
            tc.tile_pool(name="qkv", bufs=1) as qkv,
            tc.tile_pool(name="work", bufs=2) as work,
            tc.tile_pool(name="stats", bufs=8) as stats,
            tc.tile_pool(name="ps_big", bufs=2, space="PSUM") as ps_big,
            tc.tile_pool(name="ps_tp", bufs=2, space="PSUM") as ps_tp,
            tc.tile_pool(name="ps_acc", bufs=2, space="PSUM") as ps_acc,
        ):
            ident_f = cpool.tile([P, P], F32)
            make_identity(nc, ident_f)
            ident_b = cpool.tile([P, P], BF16)
            make_identity(nc, ident_b)
            ones = cpool.tile([1, 512], F32R)
            nc.gpsimd.memset(ones.bitcast(mybir.dt.uint32), 0x3F800000)
            zbias = cpool.tile([P, 1], F32)
            nc.gpsimd.memset(zbias, 0.0)

            wq_sb = cpool.tile([P, KT_TILES, DPC], F32R)
            nc.sync.dma_start(wq_sb, wq_d.rearrange("(kt p) m -> p kt m", p=P))
            wk_sb = cpool.tile([P, KT_TILES, DPC], F32R)
            nc.sync.dma_start(wk_sb, wk_d.rearrange("(kt p) m -> p kt m", p=P))
            wv_sb = cpool.tile([P, KT_TILES, DPC], F32R)
            nc.sync.dma_start(wv_sb, wv_d.rearrange("(kt p) m -> p kt m", p=P))
            wo_sb = cpool.tile([P, 2, D_MODEL], F32R)
            nc.sync.dma_start(wo_sb, wo_d.rearrange("(kt p) m -> p kt m", p=P))
            bq_sb = cpool.tile([1, DPC], F32R)
            nc.sync.dma_start(bq_sb, bq_d)
            bk_sb = cpool.tile([1, DPC], F32R)
            nc.sync.dma_start(bk_sb, bk_d)
            bv_sb = cpool.tile([1, DPC], F32R)
            nc.sync.dma_start(bv_sb, bv_d)

            QT = qkv.tile([P, 2, S], F32R)   # [d%128, d//128, q]
            KT = qkv.tile([P, 2, S], F32R)
            Vsb = qkv.tile([P, ST_TILES, DPC], BF16)  # [s%128, s//128, d]
            ctxT = qkv.tile([P, 2, S], F32R)

            # ---- Q/K projections, feature-major: dst[d, q] = W.T @ xT + b ----
            # kt-outer order: one LDWEIGHTS per (proj, m, kt), 4 matmuls per load
            for xdram, w_sb, b_sb, dst in (
                (xqT_d, wq_sb, bq_sb, QT),
                (xkT_d, wk_sb, bk_sb, KT),
            ):
                xts = []
                for kt in range(KT_TILES):
                    xt_t = xpool.tile([P, S], F32R, tag="b8k", name=f"xt{kt}")
                    nc.sync.dma_start(xt_t, xdram[kt * P : (kt + 1) * P, :])
                    xts.append(xt_t)
                for m in range(2):
                    pss = [
                        ps_big.tile([P, 1024], F32, tag="big", name="proj_ps")
                        for _ in range(2)
                    ]
                    for kt in range(KT_TILES):
                        for nn in range(2):
                            for sub in range(2):
                                c0 = nn * 1024 + sub * 512
                                nc.tensor.matmul(
                                    pss[nn][:, sub * 512 : (sub + 1) * 512],
                                    w_sb[:, kt, m * P : (m + 1) * P],
                                    xts[kt][:, c0 : c0 + 512],
                                    start=(kt == 0),
                                    stop=False,
                                )
                    for nn in range(2):
                        for sub in range(2):
                            nc.tensor.matmul(
                                pss[nn][:, sub * 512 : (sub + 1) * 512],
                                b_sb[:, m * P : (m + 1) * P],
                                ones,
                                start=False,
                                stop=True,
                            )
                        nc.any.tensor_copy(
                            dst[:, m, nn * 1024 : (nn + 1) * 1024], pss[nn]
                        )

            # ---- V projection, seq-major bf16: V[s, d] = xT.T @ Wv + bv ----
            xts = []
            for kt in range(KT_TILES):
                xt_t = xpool.tile([P, S], F32R, tag="b8k", name=f"xv{kt}")
                nc.sync.dma_start(xt_t, xvT_d[kt * P : (kt + 1) * P, :])
                xts.append(xt_t)
            for st in range(ST_TILES):
                ps = ps_big.tile([P, 1024], F32, tag="big", name="v_ps")
                psv = ps[:, :DPC]
                for kt in range(KT_TILES):
                    nc.tensor.matmul(
                        psv,
                        xts[kt][:, st * P : (st + 1) * P],
                        wv_sb[:, kt, :],
                        start=(kt == 0),
                        stop=False,
                    )
                nc.tensor.matmul(
                    psv, ones[:, :P], bv_sb, start=False, stop=True
                )
                nc.any.tensor_copy(Vsb[:, st, :], psv)

            # ---- attention per head ----
            for h in range(HPC):
                ti, pr = h // 2, (h % 2) * DK
                att_tiles = None
                invrow = work.tile([1, S], F32R, tag="invrow", name="invrow", bufs=1)
                for qt in range(ST_TILES):
                    ex = work.tile([P, S], BF16, tag="ex", name="ex")
                    rs2 = stats.tile([P, 2], F32, tag="rs2", name="rs2")
                    for half in range(2):
                        ps = ps_big.tile([P, 1024], F32, tag="big", name="sc_ps")
                        for sub in range(2):
                            col = half * 1024 + sub * 512
                            nc.tensor.matmul(
                                ps[:, sub * 512 : (sub + 1) * 512],
                                QT[pr : pr + DK, ti, qt * P : (qt + 1) * P],
                                KT[pr : pr + DK, ti, col : col + 512],
                                start=True,
                                stop=True,
                            )
                        nc.scalar.activation(
                            ex[:, half * 1024 : (half + 1) * 1024],
                            ps,
                            AF.Exp,
                            bias=zbias,
                            scale=0.125,
                            accum_out=rs2[:, half : half + 1],
                        )
                    rsum = stats.tile([P, 1], F32, tag="rs1", name="rsum")
                    nc.vector.reduce_sum(rsum, rs2, axis=mybir.AxisListType.X)
                    inv = stats.tile([P, 1], F32, tag="inv", name="inv")
                    nc.vector.reciprocal(inv, rsum)
                    # normalized bf16 attn row-block -> DRAM
                    at = work.tile([P, S], BF16, tag="at", name="at")
                    nc.vector.tensor_scalar_mul(at, ex, inv)
                    nc.sync.dma_start(attn_d[h, qt * P : (qt + 1) * P, :], at)
                    # 1/rsum as a row: PE-transpose [128,1] -> [1,128]
                    invt = ps_tp.tile([1, P], F32, tag="tp", name="invt")
                    nc.tensor.transpose(invt, inv, ident_f)
                    nc.any.tensor_copy(invrow[:, qt * P : (qt + 1) * P], invt)

                    # transpose the *unnormalized* bf16 exp into attnT tiles
                    if qt % 4 == 0:
                        att_tiles = [
                            xpool.tile([P, 4, 512], BF16, tag="b8k", name=f"attT{jj}")
                            for jj in range(4)
                        ]
                    c0 = (qt % 4) * P
                    for jj in range(4):
                        tp = ps_tp.tile([P, 512], BF16, tag="tp", name="tp")
                        for j2 in range(4):
                            j = jj * 4 + j2
                            nc.tensor.transpose(
                                tp[:, j2 * P : (j2 + 1) * P],
                                ex[:, j * P : (j + 1) * P],
                                ident_b,
                            )
                        nc.any.tensor_copy(
                            att_tiles[jj][:, :, c0 : c0 + P],
                            tp.rearrange("p (a b) -> p a b", a=4),
                        )

                    if qt % 4 == 3:
                        qc = qt // 4
                        # bc[d, q] = 1/rsum broadcast along d via PE outer product
                        bc_ps = ps_acc.tile([DK, 512], F32, tag="acc", name="bc_ps")
                        nc.tensor.matmul(
                            bc_ps,
                            ones[:1, :DK],
                            invrow[:, qc * 512 : (qc + 1) * 512],
                            start=True,
                            stop=True,
                        )
                        bc_sb = stats.tile([DK, 512], F32, tag="bc", name="bc_sb", bufs=2)
                        nc.any.tensor_copy(bc_sb, bc_ps)
                        cps = ps_acc.tile([DK, 512], F32, tag="acc", name="cps")
                        for j in range(ST_TILES):
                            nc.tensor.matmul(
                                cps,
                                Vsb[:, j, h * DK : (h + 1) * DK],
                                att_tiles[j // 4][:, j % 4, :],
                                start=(j == 0),
                                stop=(j == ST_TILES - 1),
                            )
                        nc.vector.tensor_tensor(
                            ctxT[pr : pr + DK, ti, qc * 512 : (qc + 1) * 512],
                            cps,
                            bc_sb,
                            mybir.AluOpType.mult,
                        )

            # ---- output projection: out[q, :] = ctx @ Wo ----
            for qt in range(ST_TILES):
                ob = work.tile([P, D_MODEL], F32, tag="ob", name="ob", bufs=1)
                pss = [
                    ps_acc.tile([P, 512], F32, tag="acc", name="o_ps")
                    for _ in range(2)
                ]
                for kt2 in range(2):
                    for nco in range(2):
                        nc.tensor.matmul(
                            pss[nco],
                            ctxT[:, kt2, qt * P : (qt + 1) * P],
                            wo_sb[:, kt2, nco * 512 : (nco + 1) * 512],
                            start=(kt2 == 0),
                            stop=(kt2 == 1),
                        )
                for nco in range(2):
                    nc.any.tensor_copy(ob[:, nco * 512 : (nco + 1) * 512], pss[nco])
                nc.sync.dma_start(out_d[qt * P : (qt + 1) * P, :], ob)

    nc.compile()
    return nc


_NC_CACHE = {}


def _get_nc():
    if "nc" not in _NC_CACHE:
        _NC_CACHE["nc"] = _build_nc()
    return _NC_CACHE["nc"]


def make_in_maps(query, key, value, Wq, bq, Wk, bk, Wv, bv, Wo, bo=None):
    query = np.asarray(query, np.float32)
    key = np.asarray(key, np.float32)
    value = np.asarray(value, np.float32)
    Wq = np.asarray(Wq, np.float32)
    Wk = np.asarray(Wk, np.float32)
    Wv = np.asarray(Wv, np.float32)
    Wo = np.asarray(Wo, np.float32)
    bq = np.asarray(bq, np.float32)
    bk = np.asarray(bk, np.float32)
    bv = np.asarray(bv, np.float32)

    xT = {}
    for b in range(2):
        xT[b] = (
            np.ascontiguousarray(query[b].T),
            np.ascontiguousarray(key[b].T),
            np.ascontiguousarray(value[b].T),
        )
    in_maps = []
    for c in range(8):
        b, g = divmod(c, 4)
        ds = slice(g * DPC, (g + 1) * DPC)
        xq, xk, xv = xT[b]
        in_maps.append(
            {
                "xqT": xq,
                "xkT": xk,
                "xvT": xv,
                "wq": np.ascontiguousarray(Wq[:, ds]),
                "wk": np.ascontiguousarray(Wk[:, ds]),
                "wv": np.ascontiguousarray(Wv[:, ds]),
                "wo": np.ascontiguousarray(Wo[ds, :]),
                "bq": np.ascontiguousarray(bq[ds]).reshape(1, DPC),
                "bk": np.ascontiguousarray(bk[ds]).reshape(1, DPC),
                "bv": np.ascontiguousarray(bv[ds]).reshape(1, DPC),
            }
        )
    return in_maps


def assemble(results, bo):
    bo = np.asarray(bo, np.float32)
    attn = np.empty((2, NHEAD, S, S), np.float32)
    out = np.zeros((2, S, D_MODEL), np.float32)
    for c in range(8):
        b, g = divmod(c, 4)
        attn[b, g * HPC : (g + 1) * HPC] = np.asarray(
            results[c]["attn_o"], np.float32
        )
        out[b] += results[c]["out_o"]
    out += bo
    return out, attn


def run(trace=False, trace_cores=None, **inputs):
    nc = _get_nc()
    in_maps = make_in_maps(**{k: v for k, v in inputs.items() if k != "bo"})
    res = run_bass_kernel_spmd(
        nc,
        in_maps,
        core_ids=list(range(8)),
        trace=trace,
        trace_cores=trace_cores,
    )
    out, attn = assemble(res.results, inputs["bo"])
    return out, attn, res


def kernel(query, key, value, Wq, bq, Wk, bk, Wv, bv, Wo, bo):
    out, attn, _ = run(
        query=query, key=key, value=value, Wq=Wq, bq=bq, Wk=Wk, bk=bk,
        Wv=Wv, bv=bv, Wo=Wo, bo=bo,
    )
    return out, attn
